# revision 3
# baseline (speedup 1.0000x reference)
import sys
for _p in ("/opt/trn_rl_repo",):
    if _p not in sys.path:
        sys.path.insert(0, _p)
"""Bass/Tile multi-head attention for TRN2, batch+head-group sharded, 8 cores.

Sharding: core c -> batch b = c//4, head group g = c%4 (heads 4g..4g+3),
processed as 2 phases of 2 heads each. Per phase p the core projects its
batch's q/k/v (SBUF-cached fp16 [128, 8, 2048]) through the 128-dim weight
slice for heads {2g'+...}, runs attention for those 2 heads, and accumulates
a partial output projection Wo[:, dd_p].T @ concat_p -> [1024, 2048] fp16,
stored per phase. Host sums the 16 partials (8 cores x 2 phases; 4+4 per
batch) and adds bo.

Math per phase (dims d on partitions, rows r on free axis):
  QT/KT/VT = W[dd_p] @ x^T + b        # [128, 2048] fp16, heads h0|h1 stacked
  vaug[h]  = [V_h^T | 1] per k-tile   # [128, 8, 2, 80] fp8e4 (DoubleRow kpair)
  scoresT  = K_h Q_h^T / 8            # [128 k, 2, 512] per (h, kpair), dk=64
                                      # h0/h1 use PE row groups 0/1 (row tiling)
  at       = exp(scoresT)             # fp8e4; no max-sub (scores ~ N(0,1))
  po[h]   += vaug[h][kp]^T @ at       # DoubleRow fp8: 2 k-tiles per matmul
  ccp      = po[:64]/Z packed [128, 512] fp16 (Z = po row 64, bcast via ones mm)
  out_p    = wo_p^T @ ccp             # [128ct, 512] x8, 128-dim contraction
"""
import numpy as np
import ml_dtypes

import concourse.bass as bass
import concourse.bacc as bacc
import concourse.mybir as mybir
import concourse.tile as tile

F32 = mybir.dt.float32
F16 = mybir.dt.float16
FP8 = mybir.dt.float8e4
EXP = mybir.ActivationFunctionType.Exp
DRMODE = mybir.MatmulPerfMode.DoubleRow

P = 128
DK = 64
D = 1024
DO = 8            # contraction tiles for projections
NCORES = 8
S = 2048          # rows per batch (= per core)
RBLK = 512
NRB = S // RBLK   # 4 r-blocks per phase
NKT = S // P      # 16 k-tiles
NKP = NKT // 2    # 8 k-tile pairs (DoubleRow)
NCT = D // P      # 8 output column tiles
NPH = 2           # phases (head pairs) per core
VA_W = P          # vaug row width: full 128 weight columns per k-tile.
                  # h0: V dims at cols 0-63, ones at col 64 (Z -> po row 64).
                  # h1: V dims at cols 64-127, ones at col 32 (Z -> po row 32).
                  # Full-width lhsT enables FWL and puts h1 dims on PSUM
                  # partitions 64-127 so the out-projection contracts 128.
ZROW = (DK, 32)   # Z row index in po[h] per head
EXP_SHIFT = -4.0  # exp(s/8 + EXP_SHIFT): keeps at in fp8e4m3 range (measured
                  # max s/8 = 9.02 -> max at ~ 151 < 448); cancels in po/Z


def build_kernel(reps=1, attn_dt="fp16", scores_hmajor=False,
                 ablate=()):
    ADT = FP8 if attn_dt == "fp8" else F16
    nc = bacc.Bacc("TRN2", target_bir_lowering=False, debug=False,
                   num_devices=NCORES)

    def din(name, shape, dt=F16):
        return nc.dram_tensor(name, shape, dt, kind="ExternalInput").ap()

    qT = din("qT", [D, S])
    kT = din("kT", [D, S])
    vT = din("vT", [D, S])
    wq = din("wq", [D, NPH * P])
    wk = din("wk", [D, NPH * P])
    wv = din("wv", [D, NPH * P])
    wo = din("wo", [NPH, P, D])
    bqc = din("bqc", [P, NPH], F32)
    bkc = din("bkc", [P, NPH], F32)
    bvc = din("bvc", [P, NPH], F32)
    onesZ = din("onesZ", [DK + 1, DK])
    ident = din("ident", [P, P])
    outT = nc.dram_tensor("outT", [NPH, D, S], F16, kind="ExternalOutput").ap()

    qT_r = qT.rearrange("(do p) s -> p do s", p=P)
    kT_r = kT.rearrange("(do p) s -> p do s", p=P)
    vT_r = vT.rearrange("(do p) s -> p do s", p=P)
    wq_r = wq.rearrange("(do p) c -> p do c", p=P)
    wk_r = wk.rearrange("(do p) c -> p do c", p=P)
    wv_r = wv.rearrange("(do p) c -> p do c", p=P)
    wo_r = wo.rearrange("ph d (ct c) -> d ph ct c", c=P)
    outT_r = outT.rearrange("ph (ct p) s -> p ph ct s", p=P)

    def sl(rb):
        return slice(rb * RBLK, (rb + 1) * RBLK)

    with tile.TileContext(nc) as tc:
        with tc.tile_pool(name="const", bufs=1) as const, \
             tc.tile_pool(name="mid", bufs=3) as mid, \
             tc.tile_pool(name="qtp", bufs=3) as qtp, \
             tc.tile_pool(name="atp", bufs=6) as atp, \
             tc.tile_pool(name="opp", bufs=3) as opp, \
             tc.tile_pool(name="ps_main", bufs=2, space="PSUM") as ps_main, \
             tc.tile_pool(name="ps_sc", bufs=2, space="PSUM") as ps_sc, \
             tc.tile_pool(name="ps_out", bufs=1, space="PSUM") as ps_out:

            qc = const.tile([P, DO, S], F16, tag="qc")
            kc = const.tile([P, DO, S], F16, tag="kc")
            vc = const.tile([P, DO, S], F16, tag="vc")
            wq_sb = const.tile([P, DO, NPH * P], F16, tag="wq")
            wk_sb = const.tile([P, DO, NPH * P], F16, tag="wk")
            wv_sb = const.tile([P, DO, NPH * P], F16, tag="wv")
            wo_sb = const.tile([P, NPH, NCT, P], F16, tag="wo")
            bq_sb = const.tile([P, NPH], F32, tag="bq")
            bk_sb = const.tile([P, NPH], F32, tag="bk")
            bv_sb = const.tile([P, NPH], F32, tag="bv")
            onesZ_sb = const.tile([DK + 1, DK], F16, tag="onesZ")
            ebias = const.tile([P, 1], F32, tag="ebias")
            id_sb = const.tile([P, P], F16, tag="ident")
            KT_sb = [const.tile([P, S], F16, tag=f"kt{ph}", name=f"kt{ph}")
                     for ph in range(NPH)]
            vaug = [[const.tile([P, NKP, 2, VA_W], ADT, tag=f"va{ph}{h}",
                                name=f"va{ph}{h}") for h in (0, 1)]
                    for ph in range(NPH)]

            def const_dmas():
                nc.sync.dma_start(wk_sb[:], wk_r)
                nc.sync.dma_start(bk_sb[:], bkc)
                nc.sync.dma_start(wv_sb[:], wv_r)
                nc.sync.dma_start(bv_sb[:], bvc)
                nc.sync.dma_start(wq_sb[:], wq_r)
                nc.sync.dma_start(bq_sb[:], bqc)
                nc.sync.dma_start(id_sb[:], ident)
                nc.sync.dma_start(onesZ_sb[:], onesZ)
                nc.sync.dma_start(wo_sb[:], wo_r)
                nc.vector.memset(ebias[:], EXP_SHIFT)

            def load_caches():
                # interleaved so A(0,rb) can start as soon as its chunks land
                for rb in range(NRB):
                    nc.sync.dma_start(kc[:, :, sl(rb)], kT_r[:, :, sl(rb)])
                    nc.sync.dma_start(vc[:, :, sl(rb)], vT_r[:, :, sl(rb)])
                    nc.sync.dma_start(qc[:, :, sl(rb)], qT_r[:, :, sl(rb)])

            # Warm the PE (HAM clock gate) on loaded weight data while input
            # DMAs stream; results discarded.
            def pe_warmup():
                wps = ps_main.tile([P, P], F32, tag="proj", name="warm")
                for i in range(12):
                    nc.tensor.matmul(wps[:], wk_sb[:, i % DO, 0:P],
                                     wk_sb[:, (i + 1) % DO, 0:P],
                                     start=True, stop=True)
                # Trigger the exp table load (~2.7us) during startup too.
                wsc = mid.tile([1, 16], F32, tag="wsc", name="wsc")
                nc.scalar.activation(wsc[:], id_sb[0:1, 0:16], EXP)

            # out[p_out, r] = w.T @ xT-block; contraction over D via DO tiles.
            def proj_ps(cache, w_sb, ph, rb):
                ps = ps_main.tile([P, RBLK], F32, tag="proj", name="pj")
                for do in range(DO):
                    nc.tensor.matmul(ps[:], w_sb[:, do, ph * P:(ph + 1) * P],
                                     cache[:, do, sl(rb)],
                                     start=(do == 0), stop=(do == DO - 1))
                return ps

            # ---- Stage A: K/V projections + V transpose into vaug ----
            def stage_a_parts(ph, rb):
                parts = []

                def kpart():
                    ps_k = proj_ps(kc, wk_sb, ph, rb)
                    nc.vector.tensor_scalar_add(
                        KT_sb[ph][:, sl(rb)], ps_k[:], bk_sb[:, ph:ph + 1])
                parts.append(kpart)

                vt_box = []

                def vpart():
                    ps_v = proj_ps(vc, wv_sb, ph, rb)
                    vt = mid.tile([P, RBLK], F16, tag="vt", name="vt")
                    nc.vector.tensor_scalar_add(
                        vt[:], ps_v[:], bv_sb[:, ph:ph + 1])
                    vt_box.append(vt)
                parts.append(vpart)

                def tpart(rc):
                    def f():
                        vt = vt_box[0]
                        ki = rb * (RBLK // P) + rc
                        kp, ko = ki >> 1, ki & 1
                        tp = ps_main.tile([P, P], F16, tag="proj", name="tp")
                        nc.tensor.transpose(
                            tp[:], vt[:, rc * P:(rc + 1) * P], id_sb[:])
                        nc.vector.tensor_copy(
                            vaug[ph][0][:, kp, ko, 0:DK], tp[:, 0:DK])
                        nc.vector.tensor_copy(
                            vaug[ph][1][:, kp, ko, DK:P], tp[:, DK:P])
                    return f
                for rc in range(RBLK // P):
                    parts.append(tpart(rc))
                return parts

            # ---- Stage Q ----
            qtbs = {}

            def qpart(ph, rb):
                def f():
                    ps_q = proj_ps(qc, wq_sb, ph, rb)
                    qtb = qtp.tile([P, RBLK], F16, tag="qtb", name="qtb")
                    nc.vector.tensor_scalar_add(
                        qtb[:], ps_q[:], bq_sb[:, ph:ph + 1])
                    qtbs[(ph, rb)] = qtb
                return f

            # ---- Stage B: attention ki-pair loop ----
            pos = {}

            def emit_scexp(ph, kp, qtb):
                scps = [ps_sc.tile([P, 2, RBLK], F32, tag="sc", name=f"sc{h}")
                        for h in (0, 1)]
                # ko-major issue: adjacent h0/h1 matmuls sit on disjoint PE
                # row groups (partitions 0-63 vs 64-127) and overlap 2x.
                # (scores_hmajor=True removes the adjacency, as an A/B probe
                # of whether row-group overlap actually happens on HW.)
                order = ([(h, ko) for h in (0, 1) for ko in (0, 1)]
                         if scores_hmajor else
                         [(h, ko) for ko in (0, 1) for h in (0, 1)])
                if "scores" not in ablate or kp == 0:
                    for h, ko in order:
                        ki = 2 * kp + ko
                        hs = slice(h * DK, (h + 1) * DK)
                        nc.tensor.matmul(
                            scps[h][:, ko],
                            KT_sb[ph][hs, ki * P:(ki + 1) * P],
                            qtb[hs, :], start=True, stop=True)
                ats = []
                for h in (0, 1):
                    if h == 1 and "exp1" in ablate:
                        ats.append(ats[0])
                        break
                    at = atp.tile([P, 2, RBLK], ADT, tag="at", name=f"at{h}")
                    # bias -2.5 keeps exp within fp8e4m3 range (max 448) for
                    # tail scores; the e^-2.5 factor cancels in po/Z exactly.
                    nc.scalar.activation(at[:], scps[h][:], EXP,
                                         bias=ebias[:], scale=0.125)
                    ats.append(at)
                return ats

            def emit_attnv(ph, po, kp, ats):
                if "attnv" in ablate:
                    if kp > 0:
                        return
                    for h in (0, 1):
                        nc.tensor.matmul(po[h][:], vaug[ph][h][:, 0, 0, :],
                                         ats[h][:, 0], start=True, stop=True)
                    return
                for h in (0, 1):
                    if ADT is FP8:
                        nc.tensor.matmul(
                            po[h][:], vaug[ph][h][:, kp, :, :],
                            ats[h][:],
                            start=(kp == 0), stop=(kp == NKP - 1),
                            perf_mode=DRMODE)
                    else:
                        for ko in (0, 1):
                            nc.tensor.matmul(
                                po[h][:], vaug[ph][h][:, kp, ko, :],
                                ats[h][:, ko],
                                start=(kp == 0 and ko == 0),
                                stop=(kp == NKP - 1 and ko == 1))

            def stage_b(ph, rb, prep_pre, prep):
                qtb = qtbs.pop((ph, rb))
                for pfn in prep_pre:
                    pfn()
                po = [ps_out.tile([P, RBLK], F32, tag=f"po{h}",
                                  name=f"po{h}") for h in (0, 1)]
                pos[(ph, rb)] = po
                pi = 0
                # sc/exp for kp+1 precede attnV for kp in the PE FIFO, so ACT
                # never waits on a po-fenced score matmul.
                pend = emit_scexp(ph, 0, qtb)
                for kp in range(NKP):
                    if kp >= NKP - 2:
                        # Same-cycle A-parts write KT/vaug data that the last
                        # two kp steps read; drain all prep before emitting
                        # those reads (emission order = dependency order).
                        while pi < len(prep):
                            prep[pi]()
                            pi += 1
                    nxt = emit_scexp(ph, kp + 1, qtb) if kp + 1 < NKP else None
                    emit_attnv(ph, po, kp, pend)
                    pend = nxt
                    for _ in range(3):
                        if pi < len(prep):
                            prep[pi]()
                            pi += 1
                for pfn in prep[pi:]:
                    pfn()

            # ---- Stage N: normalize + out-projection + store ----
            # po[0] rows: 0-63 = h0 dims, 64 = Z0. po[1]: 32 = Z1, 64-127 =
            # h1 dims. ccp packs both heads' normalized dims [128, RBLK] so
            # the out-projection contracts over 128 in one matmul per ct.
            def stage_n_parts(ph, rb):
                parts = []
                osbs = []
                ccp_box = []
                op_box = []

                def ncopy(h):
                    def f():
                        po = pos[(ph, rb)][h]
                        osb = mid.tile([P, RBLK], F16, tag=f"osb{h}",
                                       name=f"osb{h}")
                        nc.vector.tensor_copy(osb[:], po[:])
                        osbs.append(osb)
                    return f
                parts.append(ncopy(0))
                parts.append(ncopy(1))

                def npart():
                    zbp = ps_main.tile([P, RBLK], F32, tag="proj", name="zbp")
                    for h, hs in ((0, slice(0, DK)), (1, slice(DK, P))):
                        z = ZROW[h]
                        nc.tensor.matmul(zbp[hs, :], onesZ_sb[z:z + 1, :],
                                         osbs[h][z:z + 1, :],
                                         start=True, stop=True)
                    zrp = mid.tile([P, RBLK], F32, tag="zrp", name="zrp")
                    nc.vector.reciprocal(zrp[:], zbp[:])
                    ccp = mid.tile([P, RBLK], F16, tag="ccp", name="ccp")
                    nc.vector.tensor_mul(ccp[0:DK, :], osbs[0][0:DK, :],
                                         zrp[0:DK, :])
                    nc.vector.tensor_mul(ccp[DK:P, :], osbs[1][DK:P, :],
                                         zrp[DK:P, :])
                    ccp_box.append(ccp)
                    op_box.append(opp.tile([P, NCT, RBLK], F16, tag="op",
                                           name="op"))
                parts.append(npart)

                def oppart(ct):
                    def f():
                        op = ps_main.tile([P, RBLK], F32, tag="proj", name="op")
                        nc.tensor.matmul(op[:], wo_sb[:, ph, ct],
                                         ccp_box[0][:], start=True, stop=True)
                        if "opcopy" not in ablate or ct == 0:
                            nc.vector.tensor_copy(op_box[0][:, ct], op[:])
                    return f
                for ct in range(NCT):
                    parts.append(oppart(ct))

                def store():
                    del pos[(ph, rb)]
                    if "no_out" in ablate and not (ph == 0 and rb == 0):
                        return
                    nc.gpsimd.dma_start(outT_r[:, ph, :, sl(rb)], op_box[0][:])
                parts.append(store)
                return parts

            # ---- Schedule ----
            NCYC = NPH * NRB
            for rep in range(reps):
                if rep == 0:
                    const_dmas()
                if "cache_once" not in ablate or rep == 0:
                    load_caches()
                if rep == 0:
                    for ph in range(NPH):
                        for h in (0, 1):
                            nc.vector.memset(vaug[ph][h][:], 0.0)
                            z = ZROW[h]
                            nc.vector.memset(
                                vaug[ph][h][:, :, :, z:z + 1], 1.0)
                if rep == 0:
                    pe_warmup()
                for f in stage_a_parts(0, 0):
                    f()
                qpart(0, 0)()
                qpart(0, 1)()
                # Interleave budget note: a prep closure must be EMITTED
                # before any instruction that reads its outputs — the tile
                # framework orders by emission. With 3 slots per kp step,
                # A(0,2)+A(0,3) inside cyc 0 and A(1,rb) inside cyc rb+1
                # land ahead of their first readers.
                for cyc in range(NCYC):
                    ph, rb = divmod(cyc, NRB)
                    prep_pre, prep = [], []
                    if cyc >= 1:
                        nparts = stage_n_parts(*divmod(cyc - 1, NRB))
                        prep_pre = nparts[:2]
                        prep += nparts[2:]
                    if cyc == 0:
                        for rr in range(1, NRB):
                            prep += stage_a_parts(0, rr)
                    if 1 <= cyc <= NRB:
                        prep += stage_a_parts(1, cyc - 1)
                    if cyc + 2 < NCYC:
                        prep.append(qpart(*divmod(cyc + 2, NRB)))
                    stage_b(ph, rb, prep_pre, prep)
                for f in stage_n_parts(NPH - 1, NRB - 1):
                    f()

    nc.compile()
    return nc


def host_prepare(q, k, v, Wq, bq, Wk, bk, Wv, bv, Wo, bo):
    """Build per-core in_maps. Returns (in_maps, postprocess)."""
    f16 = np.float16
    f32 = np.float32
    q, k, v = (np.asarray(x, f32) for x in (q, k, v))
    Wq, Wk, Wv, Wo = (np.asarray(x, f32) for x in (Wq, Wk, Wv, Wo))
    bqa, bka, bva, boa = (np.asarray(x, f32) for x in (bq, bk, bv, bo))
    B = q.shape[0]
    qb = [np.ascontiguousarray(q[b].T).astype(f16) for b in range(B)]
    kb = [np.ascontiguousarray(k[b].T).astype(f16) for b in range(B)]
    vb = [np.ascontiguousarray(v[b].T).astype(f16) for b in range(B)]
    onesZ = np.ones((DK + 1, DK), f16)
    identity = np.eye(P, dtype=f16)

    in_maps = []
    for c in range(NCORES):
        b, g = divmod(c, 4)
        dd = slice(NPH * P * g, NPH * P * (g + 1))
        wo_c = np.stack([
            np.ascontiguousarray(Wo[:, NPH * P * g + P * p:
                                    NPH * P * g + P * (p + 1)].T)
            for p in range(NPH)]).astype(f16)
        in_maps.append({
            "qT": qb[b], "kT": kb[b], "vT": vb[b],
            "wq": np.ascontiguousarray(Wq[dd].T).astype(f16),
            "wk": np.ascontiguousarray(Wk[dd].T).astype(f16),
            "wv": np.ascontiguousarray(Wv[dd].T).astype(f16),
            "wo": wo_c,
            "bqc": np.ascontiguousarray(bqa[dd].reshape(NPH, P).T).astype(f32),
            "bkc": np.ascontiguousarray(bka[dd].reshape(NPH, P).T).astype(f32),
            "bvc": np.ascontiguousarray(bva[dd].reshape(NPH, P).T).astype(f32),
            "onesZ": onesZ, "ident": identity,
        })

    def postprocess(results):
        outs = []
        for b in range(B):
            acc = np.zeros((D, S), f32)
            for c in range(4 * b, 4 * b + 4):
                o = results[c]["outT"]
                acc += o[0].astype(f32)
                acc += o[1].astype(f32)
            outs.append(acc.T + boa)
        return np.stack(outs).astype(f32)

    return in_maps, postprocess


# ---------------------------------------------------------------------------
# Harness entry point: full inputs in, full output out.
# ---------------------------------------------------------------------------
_BUILD_CACHE = {}


def kernel(q, k, v, Wq, bq, Wk, bk, Wv, bv, Wo, bo, mask=0, **_unused):
    from concourse import bass_utils

    nc = _BUILD_CACHE.get("nc")
    if nc is None:
        nc = build_kernel()
        _BUILD_CACHE["nc"] = nc

    in_maps, post = host_prepare(q, k, v, Wq, bq, Wk, bk, Wv, bv, Wo, bo)
    res = bass_utils.run_bass_kernel_spmd(nc, in_maps, core_ids=list(range(8)))
    return post(res.results)
